# revision 1
# baseline (speedup 1.0000x reference)
"""Causal self-attention (B=2, S=2048, E=1024, H=16) on 8 Trainium2 cores.

Sharding: core c in 0..7 handles batch b = c//4 and the 4 heads
[4*(c%4), 4*(c%4)+4).  The host pre-transposes x[b] and pre-slices the
QKV weights column-wise / Wo row-wise per core; each core computes its
heads' attention plus its partial output projection, and the host sums
the 4 partials per batch.

Device kernel (per core, everything resident in SBUF, matmul inputs in
fp16 with fp32 PSUM accumulation):
  xT [1024,2048] -> QT,KT [d,s] and V [s,d] projections, emitted as
  per-q-block "waves" interleaved into the attention stream.
  S^T tiles = matmul(lhsT=KT_blk, rhs=QT_blk): k on partitions, q on
  the free dim; head pairs target PE row groups 0-63/64-127
  back-to-back so their K=64 matmuls overlap in the PE array.
  exp on ScalarE (1/sqrt(D) folded into the activation scale); causal
  masking = never computing strictly-below-diagonal column ranges plus
  one 128x128 triangular mask multiply per diagonal block.
  P^T @ V with V augmented by a ones column (softmax denominator falls
  out of the same accumulation); normalize with reciprocal +
  PE-broadcast of l; Y = O @ Wo streamed out per q-block so output DMA
  overlaps remaining attention work.  Projection/Y chains are split
  into ~4-matmul units and woven between attention kb-steps to keep PE
  fed while ACT (exp) paces the softmax.
"""

import numpy as np
from contextlib import ExitStack

B, S, E, H, D = 2, 2048, 1024, 16, 64
N_CORES = 8
CPB = 4              # cores per batch
HL = H // CPB        # heads per core = 4
DL = HL * D          # local head dims = 256
P = 128              # partitions
EC = E // P          # 8 e-chunks
SB = S // P          # 16 s/k blocks
NQB = S // 512       # 4 q blocks of 512
MT = DL // P         # 2 row-tiles of QT/KT/OT (2 heads each)

_CACHE = {}
_EXHAUSTED = object()


def _emit(ctx, tc, xT, wq, wk, wv, wo, mask, y, loop_n=0):
    import concourse.bass as bass  # noqa: F401
    from concourse import mybir

    nc = tc.nc
    f32 = mybir.dt.float32
    f16 = mybir.dt.float16
    Exp = mybir.ActivationFunctionType.Exp

    res = ctx.enter_context(tc.tile_pool(name="res", bufs=1))
    xt_sb = res.tile([P, EC, S], f16, tag="xt")
    wq_sb = res.tile([P, EC, DL], f16, tag="wq")
    wk_sb = res.tile([P, EC, DL], f16, tag="wk")
    wv_sb = res.tile([P, EC, DL], f16, tag="wv")
    wo_sb = res.tile([P, MT, E], f16, tag="wo")
    qt_sb = res.tile([P, MT, S], f16, tag="qt")
    kt_sb = res.tile([P, MT, S], f16, tag="kt")
    vt_sb = res.tile([P, SB, HL, D + 1], f16, tag="vt")
    ot_sb = res.tile([P, MT, S], f16, tag="ot")
    mask_sb = res.tile([P, P], f16, tag="mask")
    ones_sb = res.tile([1, D], f16, tag="ones")

    mm_ps = ctx.enter_context(tc.tile_pool(name="mm", bufs=2, space="PSUM"))
    s_ps = ctx.enter_context(tc.tile_pool(name="sps", bufs=2, space="PSUM"))
    o_ps = ctx.enter_context(tc.tile_pool(name="ops", bufs=2, space="PSUM"))

    e_pool = ctx.enter_context(tc.tile_pool(name="ep", bufs=4))
    y_pool = ctx.enter_context(tc.tile_pool(name="yp", bufs=6))
    l_pool = ctx.enter_context(tc.tile_pool(name="lp", bufs=3))

    def _full_body():
        dma = nc.sync

        # ---- loads (interleaved so the first projection wave starts early) ----
        dma.dma_start(out=mask_sb[:], in_=mask[:])
        for ec in range(EC):
            dma.dma_start(out=xt_sb[:, ec, :], in_=xT[ec * P:(ec + 1) * P, :])
            dma.dma_start(out=wq_sb[:, ec, :], in_=wq[ec * P:(ec + 1) * P, :])
            dma.dma_start(out=wk_sb[:, ec, :], in_=wk[ec * P:(ec + 1) * P, :])
        for ec in range(EC):
            dma.dma_start(out=wv_sb[:, ec, :], in_=wv[ec * P:(ec + 1) * P, :])
        for dc in range(MT):
            dma.dma_start(out=wo_sb[:, dc, :], in_=wo[dc * P:(dc + 1) * P, :])
        nc.vector.memset(ones_sb[:], 1.0)
        nc.vector.memset(vt_sb[:, :, :, D:D + 1], 1.0)

        def wave_units(nb, parts=("qt", "kt", "v")):
            # QT/KT [:, :, nb-window] = (w chunk)^T @ xT ; V[4nb..4nb+3].
            # Generator yielding ~4-matmul units so fill stays fine-grained.
            srcs = []
            if "qt" in parts:
                srcs.append((wq_sb, qt_sb))
            if "kt" in parts:
                srcs.append((wk_sb, kt_sb))
            for mt in range(MT):
                for w_sb, t_sb in srcs:
                    ps = mm_ps.tile([P, 512], f32, tag="mm")
                    for ec in range(EC):
                        nc.tensor.matmul(
                            ps[:],
                            w_sb[:, ec, mt * P:(mt + 1) * P],
                            xt_sb[:, ec, nb * 512:(nb + 1) * 512],
                            start=(ec == 0), stop=(ec == EC - 1))
                        if ec == 3:
                            yield
                    nc.vector.tensor_copy(
                        t_sb[:, mt, nb * 512:(nb + 1) * 512], ps[:])
                    yield
            if "v" not in parts:
                return
            for sb in range(4 * nb, 4 * nb + 4):
                ps = mm_ps.tile([P, 512], f32, tag="mm")
                for ec in range(EC):
                    nc.tensor.matmul(
                        ps[:, 0:DL],
                        xt_sb[:, ec, sb * P:(sb + 1) * P],
                        wv_sb[:, ec, :],
                        start=(ec == 0), stop=(ec == EC - 1))
                    if ec == 3:
                        yield
                nc.vector.tensor_copy(
                    vt_sb[:, sb, :, 0:D],
                    ps[:, 0:DL].rearrange("p (h d) -> p h d", h=HL))
                yield

        def out_proj_units(qb):
            # Y[sb, :] = O[sb, :] @ wo for this q-block's 4 s-blocks
            for sb in range(4 * qb, 4 * qb + 4):
                for eb in range(E // 512):
                    yp = mm_ps.tile([P, 512], f32, tag="mm")
                    for dc in range(MT):
                        nc.tensor.matmul(
                            yp[:],
                            ot_sb[:, dc, sb * P:(sb + 1) * P],
                            wo_sb[:, dc, eb * 512:(eb + 1) * 512],
                            start=(dc == 0), stop=(dc == MT - 1))
                    yt = y_pool.tile([P, 512], f32, tag="y")
                    nc.vector.tensor_copy(yt[:], yp[:])
                    dma.dma_start(
                        out=y[sb * P:(sb + 1) * P, eb * 512:(eb + 1) * 512],
                        in_=yt[:])
                    yield

        def attention_block(qb, fill_units, n_fill, fill_frac=1.0):
            # ACT-paced; fill units (next wave / prev Y chains, ~4 matmuls
            # each) are emitted between the S pair and the PV pair of each
            # kb-step, so PE chews fill while ACT runs exp.  Heads go in
            # pairs: the pair's two S^T matmuls target PE row groups 0-63 /
            # 64-127 back-to-back, overlapping in the array.
            nkb = 4 * (qb + 1)     # causal: k blocks 0 .. nkb-1
            scale = float(1.0 / np.sqrt(D))
            nsteps = MT * nkb
            fill_steps = max(1, int(nsteps * fill_frac))
            done = 0

            def run_fill(step):
                nonlocal done
                want = min(n_fill, ((step + 1) * n_fill) // fill_steps)
                while done < want:
                    if next(fill_units, _EXHAUSTED) is _EXHAUSTED:
                        done = n_fill
                        break
                    done += 1

            step = 0
            for mt in range(MT):   # head pair (2*mt, 2*mt+1)
                op0 = o_ps.tile([P, 512], f32, tag="o")
                op1 = o_ps.tile([P, 512], f32, tag="o")
                ops = [op0, op1]
                for kb in range(nkb):
                    t = kb - 4 * qb
                    v0 = P * t if t > 0 else 0   # masked prefix of this window
                    sp = s_ps.tile([P, 1024], f32, tag="s")
                    for half in range(2):
                        dr = half * D
                        nc.tensor.matmul(
                            sp[:, half * 512 + v0:(half + 1) * 512],
                            kt_sb[dr:dr + D, mt, kb * P:(kb + 1) * P],
                            qt_sb[dr:dr + D, mt, qb * 512 + v0:(qb + 1) * 512],
                            start=True, stop=True)
                    et = e_pool.tile([P, 1024], f16, tag="e")
                    nc.scalar.activation(out=et[:, v0:], in_=sp[:, v0:],
                                         func=Exp, scale=scale)
                    if t >= 0:  # diagonal block: mask strictly-future keys
                        for half in range(2):
                            w0 = half * 512 + v0
                            nc.vector.tensor_mul(
                                et[:, w0:w0 + P], et[:, w0:w0 + P], mask_sb[:])
                    run_fill(step)   # PE fill while ACT computes this exp
                    step += 1
                    for half in range(2):
                        nc.tensor.matmul(
                            ops[half][0:D + 1, v0:],
                            vt_sb[:, kb, 2 * mt + half, :],
                            et[:, half * 512 + v0:(half + 1) * 512],
                            start=(kb == 0), stop=(kb == nkb - 1))
                # normalize: O^T[d, q] /= l[q]  (l = ones-column row of op)
                for half in range(2):
                    op = ops[half]
                    dr = half * D
                    lcp = l_pool.tile([1, 512], f16, tag="l")
                    nc.vector.tensor_copy(lcp[:], op[D:D + 1, :])
                    bc = mm_ps.tile([P, 512], f32, tag="mm")
                    nc.tensor.matmul(bc[0:D, :], ones_sb[:], lcp[:],
                                     start=True, stop=True)
                    rec = l_pool.tile([D, 512], f32, tag="rec")
                    nc.vector.reciprocal(rec[:], bc[0:D, :])
                    nc.vector.tensor_mul(
                        ot_sb[dr:dr + D, mt, qb * 512:(qb + 1) * 512],
                        op[0:D, :], rec[:])
            # drain leftover fill
            while next(fill_units, _EXHAUSTED) is not _EXHAUSTED:
                pass

        # wave(0) is DMA-paced (nothing else to run): interleave all four
        # qt/kt chains per e-chunk -- two accumulators from the mm pool,
        # two borrowed from the (still idle) o pool -- so each arriving
        # xt chunk feeds 4 matmuls and the chains complete right after
        # the last chunk lands.  V chains follow (chunks then resident).
        w0ps = []
        for mt in range(MT):
            pq = mm_ps.tile([P, 512], f32, tag="mm")
            pk = o_ps.tile([P, 512], f32, tag="o")
            w0ps.append((mt, wq_sb, qt_sb, pq))
            w0ps.append((mt, wk_sb, kt_sb, pk))
        for ec in range(EC):
            for mt, w_sb, t_sb, pchain in w0ps:
                nc.tensor.matmul(
                    pchain[:],
                    w_sb[:, ec, mt * P:(mt + 1) * P],
                    xt_sb[:, ec, 0:512],
                    start=(ec == 0), stop=(ec == EC - 1))
        for mt, w_sb, t_sb, pchain in w0ps:
            nc.vector.tensor_copy(t_sb[:, mt, 0:512], pchain[:])
        for sb in range(4):
            ps = mm_ps.tile([P, 512], f32, tag="mm")
            for ec in range(EC):
                nc.tensor.matmul(
                    ps[:, 0:DL],
                    xt_sb[:, ec, sb * P:(sb + 1) * P],
                    wv_sb[:, ec, :],
                    start=(ec == 0), stop=(ec == EC - 1))
            nc.vector.tensor_copy(
                vt_sb[:, sb, :, 0:D],
                ps[:, 0:DL].rearrange("p (h d) -> p h d", h=HL))
        # Fill plan: attention(qb) gets wave(qb+1) + Y(qb-1).  wave(3) is
        # split: its qt chains (needed at attn(3) step 0) stay in attn(2)'s
        # fill; its kt + V chains (only needed from kb=12) move into
        # attn(3)'s fill, front-loaded to land before kb=12 -- this drains
        # PE work from the PE-stuffed attn(2) into attn(3)'s ACT-paced
        # slack.  Unit counts: qt/kt chain = 2 units, V chain = 2 units.
        for qb in range(NQB):
            gens = []
            n_fill = 0
            frac = 1.0
            if qb + 1 < NQB - 1:
                gens.append(wave_units(qb + 1))
                n_fill += 16
            elif qb + 1 == NQB - 1:      # qb == 2: wave(3)'s qt + kt parts
                gens.append(wave_units(qb + 1, parts=("qt", "kt")))
                n_fill += 8
            else:                        # qb == 3: wave(3)'s V chains only
                gens.append(wave_units(qb, parts=("v",)))
                n_fill += 8
                frac = 0.55              # land before the kb=12 diagonal
            if qb > 0:
                gens.append(out_proj_units(qb - 1))
                n_fill += 8
            def _chain(gs=tuple(gens)):
                for g in gs:
                    yield from g
            attention_block(qb, _chain(), max(n_fill, 1), fill_frac=frac)
        for _ in out_proj_units(NQB - 1):
            pass

    if loop_n:
        # bench-only path: hint all engines so the back-edge prefetches
        # the body's IRAM blocks (body >256 instructions per engine)
        hints = (mybir.EngineType.PE, mybir.EngineType.Activation,
                 mybir.EngineType.DVE, mybir.EngineType.SP,
                 mybir.EngineType.Pool)
        with tc.For_i(0, loop_n, 1, hint_engines=hints):
            _full_body()
    else:
        _full_body()


def _get_program(loop_n=0):
    key = ("nc", loop_n)
    if key in _CACHE:
        return _CACHE[key]
    import concourse.tile as tile
    from concourse import bacc, mybir

    f32 = mybir.dt.float32
    f16 = mybir.dt.float16
    nc = bacc.Bacc("TRN2", target_bir_lowering=False, debug=False,
                   enable_asserts=False)
    xT = nc.dram_tensor("xT", [E, S], f16, kind="ExternalInput").ap()
    wq = nc.dram_tensor("wq", [E, DL], f16, kind="ExternalInput").ap()
    wk = nc.dram_tensor("wk", [E, DL], f16, kind="ExternalInput").ap()
    wv = nc.dram_tensor("wv", [E, DL], f16, kind="ExternalInput").ap()
    wo = nc.dram_tensor("wo", [DL, E], f16, kind="ExternalInput").ap()
    mask = nc.dram_tensor("mask", [P, P], f16, kind="ExternalInput").ap()
    y = nc.dram_tensor("y", [S, E], f32, kind="ExternalOutput").ap()
    with tile.TileContext(nc) as tc:
        with ExitStack() as ctx:
            _emit(ctx, tc, xT, wq, wk, wv, wo, mask, y, loop_n=loop_n)
    nc.compile()
    _CACHE[key] = nc
    return nc


def _make_in_maps(x, Wq, Wk, Wv, Wo):
    x = np.asarray(x, dtype=np.float32)
    Wq = np.asarray(Wq, dtype=np.float32)
    Wk = np.asarray(Wk, dtype=np.float32)
    Wv = np.asarray(Wv, dtype=np.float32)
    Wo = np.asarray(Wo, dtype=np.float32)
    mask = np.triu(np.ones((P, P), dtype=np.float16))
    in_maps = []
    for c in range(N_CORES):
        b, hg = divmod(c, CPB)
        hs = slice(hg * HL, (hg + 1) * HL)
        in_maps.append({
            "xT": np.ascontiguousarray(x[b].T).astype(np.float16),
            "wq": np.ascontiguousarray(Wq.reshape(E, H, D)[:, hs, :].reshape(E, DL)).astype(np.float16),
            "wk": np.ascontiguousarray(Wk.reshape(E, H, D)[:, hs, :].reshape(E, DL)).astype(np.float16),
            "wv": np.ascontiguousarray(Wv.reshape(E, H, D)[:, hs, :].reshape(E, DL)).astype(np.float16),
            "wo": np.ascontiguousarray(Wo.reshape(H, D, E)[hs, :, :].reshape(DL, E)).astype(np.float16),
            "mask": mask,
        })
    return in_maps


def run(x, Wq, Wk, Wv, Wo, trace=False):
    from concourse.bass_utils import run_bass_kernel_spmd

    nc = _get_program()
    in_maps = _make_in_maps(x, Wq, Wk, Wv, Wo)
    br = run_bass_kernel_spmd(nc, in_maps, list(range(N_CORES)), trace=trace)
    out = np.zeros((B, S, E), dtype=np.float32)
    for c in range(N_CORES):
        out[c // CPB] += br.results[c]["y"]
    return out, br


def kernel(x, Wq, Wk, Wv, Wo):
    out, _ = run(x, Wq, Wk, Wv, Wo, trace=False)
    return out

